# revision 2
# baseline (speedup 1.0000x reference)
"""Trainium2 Bass kernel for the analytic ellipsoid renderer (nn_AnalyticRenderer).

reference math:
  out[v,u,w] = sum_n where(disc>0, |S rn| * sqrt(disc), 0)
which algebraically reduces (ray-normalizations cancel; S @ Sinv = I) to
  out[v,u,w] = sum_n sqrt(relu(F_nv(u,w))) / q_nv(u,w)
    q  = |Sinv K pix|^2                      (quadratic bilinear form in u,w)
    F  = 4 * |K pix|^2 * ((Cn.g)^2 - ctil*q) (quartic bilinear form)
with pix=[u,w,1], K = inv(P[:, :3,:3]), and per-(n,v) constants from P,M,S.

Device strategy (8 NeuronCores, SPMD; one graph, per-core coefficient data):
  - image split into 32 row-tiles (122 rows x 976 cols); 4 tiles per core
  - per tile, up to S[j] (ellipsoid) sub-items; schedule shape shared SPMD
  - per sub-item: PE evaluates F and q via K=20/K=12 matmuls against
    hi/lo-split bf16 per-row-coefficient weights and w-power features
    (per-item basis center; ill-conditioned items use their epipole column);
    ACT computes s = Sqrt(F) (NaN where F<0); a custom fused DVE op computes
    z = relu(s) * recip_1NR(q) (relu kills the NaN mask); an fp16 identity
    matmul accumulates z into the PSUM accumulator (the sum over ellipsoids).
  - per tile: ACT copies the PSUM accumulator to SBUF, DMA to DRAM out.
"""
import sys
import os

sys.path.insert(0, "/opt/trn_rl_repo")

import numpy as np
import ml_dtypes
from math import comb

import concourse.bass as bass
import concourse.bacc as bacc
import concourse.tile as tile
import concourse.mybir as mybir
from concourse.bass_utils import run_bass_kernel_spmd

V, N, U, W = 4, 8, 976, 976
TROWS = 122
NTILES = U // TROWS
WCENTER = 487.5
RECIP_C0 = -0.23549792
RECIP_C1 = 2.0017324
ILL_THRESH = 1.5e-3
f32 = mybir.dt.float32
f16 = mybir.dt.float16
bf16 = mybir.dt.bfloat16

# --------------------------------------------------------------------------
# custom DVE op: out = relu(Src1) * recip_1nr(Src0)
# --------------------------------------------------------------------------
from concourse.dve_spec import Spec, Bin, AluOp, Src0, Src1, relu as dve_relu, C0, C1, lower
from concourse.dve_uop import DveOpSpec
import concourse.dve_ops as dve_ops
from concourse.dve_ops import DveOp


def _ref_relu_mul_recip1nr(in0, in1, c0, c1, c2):
    not_x = (~in0.view(np.int32)).view(np.float32)
    y0 = not_x * c0
    y1 = y0 * (c1 - in0 * y0)
    s = np.maximum(np.nan_to_num(in1.astype(np.float32), nan=0.0), 0.0)
    return s * y1


def _register_zop():
    name = "RELU_MUL_RECIP1NR_ANT"
    if name in dve_ops._SUB_OPCODE_FOR_NAME:
        for op in dve_ops.OPS:
            if op.name == name:
                return op
    _not_x = Bin(AluOp.BITWISE_NOT, Src0, Src0)
    _y0 = _not_x * C0
    _y1 = _y0 * (C1 - Src0 * _y0)
    spec = Spec(body=dve_relu(Src1) * _y1, reference=_ref_relu_mul_recip1nr)
    row = max(dve_ops._SUB_OPCODE_FOR_NAME.values()) + 1
    shas = {}
    for ver in ("v3", "v4"):
        try:
            uops = lower(spec, ver=ver)
            shas[ver] = DveOpSpec(name=name, opcode=row, uops=uops, rd1_en=True).sha(ver)
        except Exception:
            pass
    op = DveOp(name, spec, subdim=False, uops_sha=shas)
    dve_ops.OPS.append(op)
    dve_ops.CUSTOM_DVE_SPECS[name] = spec
    dve_ops._SUB_OPCODE_FOR_NAME[name] = row
    return op


ZOP = _register_zop()

# --------------------------------------------------------------------------
# host precompute (see derivation in module docstring)
# --------------------------------------------------------------------------


def _geometry(P, M, S):
    P64, M64, S64 = P.astype(np.float64), M.astype(np.float64), S.astype(np.float64)
    K = np.linalg.inv(P64[:, :3, :3])
    C = -np.einsum('vij,vj->vi', K, P64[:, :3, 3])
    Sinv = np.linalg.inv(S64)
    Q = np.einsum('nij,vjk->nvik', Sinv, K)
    Cn = np.einsum('nij,vnj->vni', Sinv, C[:, None, :] - M64[None, :, :])
    a_vec = np.einsum('nvji,vnj->nvi', Q, Cn)
    ctil = np.einsum('vni,vni->vn', Cn, Cn) - 1.0
    G = np.einsum('nvji,nvjk->nvik', Q, Q)
    KtK = np.einsum('vji,vjk->vik', K, K)
    return a_vec, ctil, G, KtK


def _quad_to_mat(B):
    B = 0.5 * (B + B.T)
    Mq = np.zeros((3, 3))
    Mq[2, 0] = B[0, 0]; Mq[0, 2] = B[1, 1]; Mq[0, 0] = B[2, 2]
    Mq[1, 1] = 2 * B[0, 1]; Mq[1, 0] = 2 * B[0, 2]; Mq[0, 1] = 2 * B[1, 2]
    return Mq


def _bilinear_forms(P, M, S):
    a_vec, ctil, G, KtK = _geometry(P, M, S)
    Fm = np.zeros((V, N, 5, 5)); qm = np.zeros((V, N, 3, 3))
    for v in range(V):
        rrm = _quad_to_mat(KtK[v])
        for n in range(N):
            qm[v, n] = _quad_to_mat(G[n, v])
            a = a_vec[n, v]
            dotm = np.zeros((3, 3))
            dotm[2, 0] = a[0] ** 2; dotm[0, 2] = a[1] ** 2; dotm[0, 0] = a[2] ** 2
            dotm[1, 1] = 2 * a[0] * a[1]; dotm[1, 0] = 2 * a[0] * a[2]
            dotm[0, 1] = 2 * a[1] * a[2]
            Dtm = dotm - ctil[v, n] * qm[v, n]
            Fm5 = np.zeros((5, 5))
            for i in range(3):
                for j in range(3):
                    Fm5[i:i + 3, j:j + 3] += 4.0 * rrm[i, j] * Dtm
            Fm[v, n] = Fm5
    return Fm, qm


def _shift_T(deg, c):
    T = np.zeros((deg, deg))
    for j in range(deg):
        for p in range(j + 1):
            T[j, p] = comb(j, p) * c ** (j - p)
    return T


def _split_hi_lo(x):
    x32 = np.asarray(x, dtype=np.float32)
    hi = x32.astype(ml_dtypes.bfloat16)
    lo = (x32 - hi.astype(np.float32)).astype(ml_dtypes.bfloat16)
    return hi, lo


def _feat_block(c, deg):
    wp = np.arange(W, dtype=np.float64) - c
    pows = np.stack([wp ** p for p in range(deg)], axis=0)
    hi, lo = _split_hi_lo(pows)
    return np.concatenate([hi, lo, hi, lo], axis=0)


def _pack_w(coeffs_T):
    hi, lo = _split_hi_lo(coeffs_T)
    return np.concatenate([hi, hi, lo, lo], axis=0)


def _prepare(P, M, S_in):
    Fm, qm = _bilinear_forms(P, M, S_in)
    u = np.arange(U, dtype=np.float64)
    ub5 = np.stack([u ** k for k in range(5)], axis=1)
    Fc = np.einsum('up,vnpj,jq->vnuq', ub5, Fm, _shift_T(5, WCENTER))
    qc = np.einsum('up,vnpj,jq->vnuq', ub5[:, :3], qm, _shift_T(3, WCENTER))

    wp = np.arange(W, dtype=np.float64) - WCENTER
    wb5 = np.stack([wp ** k for k in range(5)], axis=1)
    wb3 = wb5[:, :3]

    act = np.zeros((V, N, NTILES), dtype=bool)
    fmax = np.zeros((V, N, NTILES))
    qmin = np.zeros((V, N, NTILES))
    qterms = np.zeros((V, N, NTILES))
    for v in range(V):
        for n in range(N):
            Fg = (Fc[v, n] @ wb5.T).reshape(NTILES, TROWS, W)
            qg = (qc[v, n] @ wb3.T).reshape(NTILES, TROWS, W)
            act[v, n] = (Fg > 0).any(axis=(1, 2))
            fmax[v, n] = Fg.max(axis=(1, 2))
            qmin[v, n] = qg.min(axis=(1, 2))
            qt = (np.abs(qc[v, n]) * np.array([1.0, 488.0, 488.0 ** 2])).sum(axis=1)
            qterms[v, n] = qt.reshape(NTILES, TROWS).max(axis=1)
    ill = act & (qmin < qterms * ILL_THRESH)

    items = []
    for v in range(V):
        for t in range(NTILES):
            ns = [n for n in range(N) if act[v, n, t]]
            items.append(((v, t), ns))
    items.sort(key=lambda x: -len(x[1]))
    buckets = [[] for _ in range(8)]
    for i, it in enumerate(items):
        buckets[i % 8].append(it)
    S = [max(max(len(b[j][1]) for b in buckets), 1) for j in range(4)]
    SS = sum(S)
    soffs = np.cumsum([0] + S[:-1])

    wfs = np.zeros((8, 20, SS * TROWS), dtype=ml_dtypes.bfloat16)
    wqs = np.zeros((8, 12, SS * TROWS), dtype=ml_dtypes.bfloat16)
    fbankF = np.zeros((8, 20, SS * W), dtype=ml_dtypes.bfloat16)
    fbankq = np.zeros((8, 12, SS * W), dtype=ml_dtypes.bfloat16)
    slotmap = [[None] * 4 for _ in range(8)]

    featF_c = _feat_block(WCENTER, 5)
    featq_c = _feat_block(WCENTER, 3)

    for c in range(8):
        for j in range(4):
            (v, t), ns = buckets[c][j]
            slotmap[c][j] = (v, t)
            rows = np.s_[t * TROWS:(t + 1) * TROWS]
            u_abs = np.arange(t * TROWS, (t + 1) * TROWS, dtype=np.float64)
            ub5t = np.stack([u_abs ** k2 for k2 in range(5)], axis=1)
            for s in range(S[j]):
                sl = np.s_[(soffs[j] + s) * TROWS:(soffs[j] + s + 1) * TROWS]
                slw = np.s_[(soffs[j] + s) * W:(soffs[j] + s + 1) * W]
                if s < len(ns):
                    n = ns[s]
                    if ill[v, n, t]:
                        c2 = qc[v, n, rows, 2]; c1 = qc[v, n, rows, 1]
                        w0 = -c1 / (2 * c2)
                        m = qc[v, n, rows, 0] - c1 ** 2 / (4 * c2)
                        ustar = int(np.argmin(m))
                        cw = WCENTER + w0[ustar]
                        Fcc = np.einsum('up,pj,jq->uq', ub5t, Fm[v, n], _shift_T(5, cw))
                        qcc = np.einsum('up,pj,jq->uq', ub5t[:, :3], qm[v, n], _shift_T(3, cw))
                        fF = _feat_block(cw, 5); fq = _feat_block(cw, 3)
                    else:
                        Fcc = Fc[v, n, rows]; qcc = qc[v, n, rows]
                        fF = featF_c; fq = featq_c
                    fmx = max(float(np.sqrt(max(fmax[v, n, t], 1e-30))), 1e-30)
                    k = max(0.0, np.ceil(np.log2(fmx) - 12.0))
                    wfs[c, :, sl] = _pack_w((Fcc * 4.0 ** -k).T)
                    wqs[c, :, sl] = _pack_w((qcc * 2.0 ** -k).T)
                    fbankF[c, :, slw] = fF
                    fbankq[c, :, slw] = fq
                else:
                    wqs[c, 0, sl] = 1.0
                    fbankq[c, 0, slw] = 1.0
    return dict(S=S, SS=SS, soffs=soffs, wfs=wfs, wqs=wqs,
                fbankF=fbankF, fbankq=fbankq, slotmap=slotmap)


# --------------------------------------------------------------------------
# bass graph
# --------------------------------------------------------------------------


def _build_nc(S, soffs, SS):
    nc = bacc.Bacc(None, target_bir_lowering=False, debug=False)
    d_wfs = nc.declare_dram_parameter("wfs", [20, SS * TROWS], bf16, isOutput=False)
    d_wqs = nc.declare_dram_parameter("wqs", [12, SS * TROWS], bf16, isOutput=False)
    d_fbF = nc.declare_dram_parameter("fbF", [20, SS * W], bf16, isOutput=False)
    d_fbq = nc.declare_dram_parameter("fbq", [12, SS * W], bf16, isOutput=False)
    d_id = nc.declare_dram_parameter("ident", [128, 128], f16, isOutput=False)
    d_out = nc.declare_dram_parameter("out", [4, TROWS, W], f32, isOutput=True)

    with tile.TileContext(nc) as tc:
        with (
            tc.tile_pool(name="consts", bufs=1) as consts,
            tc.tile_pool(name="sz", bufs=3) as szp,
            tc.tile_pool(name="op", bufs=2) as opool,
            tc.tile_pool(name="pq", bufs=3, space="PSUM") as pqp,
            tc.tile_pool(name="pacc", bufs=1, space="PSUM") as paccp,
        ):
            t_wfs = consts.tile([20, SS * TROWS], bf16)
            nc.sync.dma_start(t_wfs[:], d_wfs[:])
            t_wqs = consts.tile([12, SS * TROWS], bf16)
            nc.sync.dma_start(t_wqs[:], d_wqs[:])
            t_fbF = consts.tile([20, SS * W], bf16)
            nc.sync.dma_start(t_fbF[:], d_fbF[:])
            t_fbq = consts.tile([12, SS * W], bf16)
            nc.sync.dma_start(t_fbq[:], d_fbq[:])
            t_id = consts.tile([128, 128], f16)
            nc.sync.dma_start(t_id[:], d_id[:])

            for j in range(4):
                acc = paccp.tile([128, 2, 512], f32, tag="acc")
                for s in range(S[j]):
                    idx = int(soffs[j]) + s
                    Ft = pqp.tile([128, 2, 512], f32, tag="fq")
                    qt = pqp.tile([128, 2, 512], f32, tag="fq")
                    for h in range(2):
                        nc.tensor.matmul(
                            Ft[0:TROWS, h, 0:488],
                            t_wfs[:, idx * TROWS:(idx + 1) * TROWS],
                            t_fbF[:, idx * W + h * 488: idx * W + (h + 1) * 488],
                            start=True, stop=True,
                        )
                        nc.tensor.matmul(
                            qt[0:TROWS, h, 0:488],
                            t_wqs[:, idx * TROWS:(idx + 1) * TROWS],
                            t_fbq[:, idx * W + h * 488: idx * W + (h + 1) * 488],
                            start=True, stop=True,
                        )
                    s_t = szp.tile([128, 2, 488], f16, tag="s")
                    nc.scalar.activation(
                        s_t[0:TROWS, :, :], Ft[0:TROWS, :, 0:488],
                        mybir.ActivationFunctionType.Sqrt,
                    )
                    z_t = szp.tile([128, 2, 488], f16, tag="z")
                    nc.vector._custom_dve(
                        ZOP, out=z_t[0:TROWS, :, :], in0=qt[0:TROWS, :, 0:488],
                        in1=s_t[0:TROWS, :, :], s0=RECIP_C0, s1=RECIP_C1,
                    )
                    for h in range(2):
                        # contract only rows 0:TROWS — rows 122..127 of z_t are
                        # uninitialized SBUF and 0*NaN would poison the column
                        nc.tensor.matmul(
                            acc[:, h, 0:488], t_id[0:TROWS, :], z_t[0:TROWS, h, :],
                            start=(s == 0), stop=(s == S[j] - 1),
                        )
                o_t = opool.tile([128, 2, 488], f32, tag="o")
                nc.scalar.copy(o_t[0:TROWS, :, :], acc[0:TROWS, :, 0:488])
                nc.sync.dma_start(
                    d_out[j].rearrange("p (h w) -> p h w", h=2), o_t[0:TROWS, :, :]
                )
    nc.compile()
    return nc


_CACHE = {}


def kernel(P, M, S):
    P = np.ascontiguousarray(np.asarray(P, dtype=np.float32))
    M = np.ascontiguousarray(np.asarray(M, dtype=np.float32))
    S = np.ascontiguousarray(np.asarray(S, dtype=np.float32))
    prep = _prepare(P, M, S)
    Ssch, soffs, SS = prep["S"], prep["soffs"], prep["SS"]

    key = tuple(Ssch)
    if key not in _CACHE:
        _CACHE[key] = _build_nc(Ssch, soffs, SS)
    nc = _CACHE[key]

    ident = np.eye(128, dtype=np.float16)
    in_maps = []
    for c in range(8):
        in_maps.append({
            "wfs": np.ascontiguousarray(prep["wfs"][c]).view(np.uint16),
            "wqs": np.ascontiguousarray(prep["wqs"][c]).view(np.uint16),
            "fbF": np.ascontiguousarray(prep["fbankF"][c]).view(np.uint16),
            "fbq": np.ascontiguousarray(prep["fbankq"][c]).view(np.uint16),
            "ident": ident,
        })
    res = run_bass_kernel_spmd(nc, in_maps, core_ids=list(range(8)))

    out = np.zeros((V, U, W), dtype=np.float32)
    for c in range(8):
        o = res.results[c]["out"]
        for j in range(4):
            v, t = prep["slotmap"][c][j]
            out[v, t * TROWS:(t + 1) * TROWS, :] = o[j]
    return out


if __name__ == "__main__":
    P = np.load(os.path.join(os.path.dirname(__file__), 'P.npy'))
    M = np.load(os.path.join(os.path.dirname(__file__), 'M.npy'))
    S = np.load(os.path.join(os.path.dirname(__file__), 'S.npy'))
    o = kernel(P=P, M=M, S=S)
    print("out", o.shape, o.dtype, float(np.linalg.norm(o)))
